# revision 25
# baseline (speedup 1.0000x reference)
"""Trainium2 Bass kernel for nn_Discriminator (LSTM + conv branch + MLP head).

Data-parallel over 8 NeuronCores: batch 512 -> 64 per core, weights replicated.

Fully transposed LSTM: gate features live on partitions, batch on the free
dim.  z^T for step t is a single PSUM bank (128 x [f0 f1 i0 i1 g0 g1 o0 o1]
x 64 batch cols), produced by 8 x-part matmuls (issued one step ahead) plus
16 recurrent bf16 matmuls whose moving dim is the 64-col batch -- the full
128x128 PE array is used, and h^T feeds the next step's matmuls directly, so
the recurrent cycle has no PE transposes and no PSUM->SBUF copies:

    PE z (16 mm) -> ACT sig(f,i) -> DVE q1,q2,c -> ACT tanh(c) -> DVE h -> PE

Conv branch / flatten-dense / MLP are unchanged from the natural-layout
version: convT = lrelu(Wc^T @ xT) in (CF x T*Bc), out2T += Wd_chunk^T @
convT_t per step, BatchNorms folded into W1/b1 on host, MLP transposed.
"""

import numpy as np

import concourse.bass as bass
import concourse.mybir as mybir
import concourse.tile as tile
from concourse import bacc, bass_utils

F32 = mybir.dt.float32
BF16 = mybir.dt.bfloat16

B, T, F, H, CF = 512, 256, 64, 256, 128
N_CORES = 8
BC = B // N_CORES  # 64
EPS = 1e-3
NT = T * BC  # 16384 columns of xT / convT

_CACHE = {}


def _build_nc():
    nc = bacc.Bacc("TRN2", target_bir_lowering=False, debug=False,
                   num_devices=N_CORES)

    d = {}
    d["xT"] = nc.dram_tensor("xT", [128, NT], BF16, kind="ExternalInput").ap()
    d["wxT"] = nc.dram_tensor("wxT", [128, 8 * 128], BF16, kind="ExternalInput").ap()
    d["whT"] = nc.dram_tensor("whT", [128, 16 * 128], BF16, kind="ExternalInput").ap()
    d["wcp"] = nc.dram_tensor("wcp", [128, CF], BF16, kind="ExternalInput").ap()
    d["wdp"] = nc.dram_tensor("wdp", [128, T * 2 * 128], BF16, kind="ExternalInput").ap()
    d["bdp"] = nc.dram_tensor("bdp", [128, 2], F32, kind="ExternalInput").ap()
    d["w1p"] = nc.dram_tensor("w1p", [128, 4 * 8 * 128], BF16, kind="ExternalInput").ap()
    d["b1bp"] = nc.dram_tensor("b1bp", [128, 8 * 128], BF16, kind="ExternalInput").ap()
    d["w2p"] = nc.dram_tensor("w2p", [128, 8 * 8 * 128], BF16, kind="ExternalInput").ap()
    d["b2bp"] = nc.dram_tensor("b2bp", [128, 8 * 128], BF16, kind="ExternalInput").ap()
    d["onesp"] = nc.dram_tensor("onesp", [128, 64], BF16, kind="ExternalInput").ap()
    d["w3p"] = nc.dram_tensor("w3p", [128, 8], BF16, kind="ExternalInput").ap()
    d["b3p"] = nc.dram_tensor("b3p", [1, 1], F32, kind="ExternalInput").ap()
    out_d = nc.dram_tensor("out", [BC, 1], F32, kind="ExternalOutput").ap()

    PRELU = mybir.ActivationFunctionType.Prelu
    SIGM = mybir.ActivationFunctionType.Sigmoid
    TANH = mybir.ActivationFunctionType.Tanh
    MUL = mybir.AluOpType.mult
    ADD = mybir.AluOpType.add

    with tile.TileContext(nc) as tc:
        with (
            tc.tile_pool(name="const", bufs=1) as const,
            tc.tile_pool(name="wds_p", bufs=2) as wds_p,
            tc.tile_pool(name="gates", bufs=2) as gates,
            tc.tile_pool(name="state", bufs=2) as state,
            tc.tile_pool(name="ps_z", bufs=2, space="PSUM") as ps_z,
            tc.tile_pool(name="ps_o2", bufs=1, space="PSUM") as ps_o2,
            tc.tile_pool(name="ps_cv", bufs=1, space="PSUM") as ps_cv,
        ):
            # DMA order = priority: t=0's f,i matmuls need wxT[:, 0:512] and
            # xT[:, 0:64]; whT is needed from t=1; the rest streams behind.
            wxT = const.tile([128, 8 * 128], BF16)
            nc.sync.dma_start(out=wxT[:, 0:512], in_=d["wxT"][:, 0:512])
            xT = const.tile([128, NT], BF16)
            nc.sync.dma_start(out=xT[:, 0:64], in_=d["xT"][:, 0:64])
            nc.sync.dma_start(out=wxT[:, 512:1024], in_=d["wxT"][:, 512:1024])
            nc.sync.dma_start(out=xT[:, 64:512], in_=d["xT"][:, 64:512])
            whT = const.tile([128, 16 * 128], BF16)
            nc.sync.dma_start(out=whT, in_=d["whT"])
            wcp = const.tile([128, CF], BF16)
            nc.sync.dma_start(out=wcp, in_=d["wcp"])
            CH = 1984
            for i in range(8):
                sl = slice(512 + i * CH, 512 + (i + 1) * CH)
                nc.sync.dma_start(out=xT[:, sl], in_=d["xT"][:, sl])
            bdp = const.tile([128, 2], F32)
            nc.sync.dma_start(out=bdp, in_=d["bdp"])
            w1p = const.tile([128, 4 * 8 * 128], BF16)
            nc.sync.dma_start(out=w1p, in_=d["w1p"])
            b1bp = const.tile([128, 8 * 128], BF16)
            nc.sync.dma_start(out=b1bp, in_=d["b1bp"])
            w2p = const.tile([128, 8 * 8 * 128], BF16)
            nc.sync.dma_start(out=w2p, in_=d["w2p"])
            b2bp = const.tile([128, 8 * 128], BF16)
            nc.sync.dma_start(out=b2bp, in_=d["b2bp"])
            onesp = const.tile([128, 64], BF16)
            nc.sync.dma_start(out=onesp, in_=d["onesp"])
            w3p = const.tile([128, 8], BF16)
            nc.sync.dma_start(out=w3p, in_=d["w3p"])
            b3p = const.tile([1, 1], F32)
            nc.sync.dma_start(out=b3p, in_=d["b3p"])

            convT = const.tile([128, NT], BF16)
            out2T = ps_o2.tile([128, 128], F32)  # [:, 0:64]=feat 0-127, [:, 64:128]=feat 128-255

            def x_mms(t, stop):
                # x-part of z^T for step t, split into two PSUM tiles (banks)
                # so the gate ACT ops only wait on their own bank's matmuls.
                # Each bank's chunk-0 start=True clears that bank's
                # has_written bits, so later chunks and the recurrent matmuls
                # overwrite-then-accumulate correctly.
                pfi = ps_z.tile([128, 256], F32, tag="pfi", name="pfi")
                pgo = ps_z.tile([128, 256], F32, tag="pgo", name="pgo")
                for j in range(8):
                    dst = pfi if j < 4 else pgo
                    nc.tensor.matmul(dst[:, (j % 4) * 64:(j % 4 + 1) * 64],
                                     wxT[:, j * 128:(j + 1) * 128],
                                     xT[:, t * BC:(t + 1) * BC],
                                     start=(j % 4 == 0), stop=stop,
                                     skip_group_check=True)
                return pfi, pgo

            def conv_chunk(ci):
                # conv-branch matmul for chunk ci (8 timesteps) + Wd stream.
                cs = slice(ci * 512, (ci + 1) * 512)
                pcv = ps_cv.tile([128, 512], F32, tag="cv", name="pcv")
                nc.tensor.matmul(pcv, wcp, xT[:, cs], start=True, stop=True)
                wds = wds_p.tile([128, 8 * 2 * 128], BF16, tag="wds", name="wds")
                nc.sync.dma_start(
                    out=wds, in_=d["wdp"][:, ci * 2048:(ci + 1) * 2048])
                return pcv, wds

            def conv_lrelu(pcv, ci, half):
                cs = slice(ci * 512 + half * 256, ci * 512 + (half + 1) * 256)
                nc.scalar.activation(convT[:, cs],
                                     pcv[:, half * 256:(half + 1) * 256],
                                     PRELU, alpha=0.2)

            pfi, pgo = x_mms(0, stop=True)
            pcv_cur, wds_cur = conv_chunk(0)
            conv_lrelu(pcv_cur, 0, 0)
            conv_lrelu(pcv_cur, 0, 1)

            hT = None   # (128, 128) bf16: [:, 0:64]=h feat 0-127 ^T, [:, 64:]=feat 128-255 ^T
            cT = None   # (128, 128) f32, same layout

            for t in range(T):
                tb = slice(t * BC, (t + 1) * BC)
                if t % 8 == 0:
                    wds = wds_cur

                # ---- recurrent z^T matmuls (x-part was issued last iter) ----
                # f,i bank first so sig(f,i) can start after 8 matmuls.
                if t > 0:
                    for j in range(8):
                        dst = pfi if j < 4 else pgo
                        for k in range(2):
                            nc.tensor.matmul(
                                dst[:, (j % 4) * 64:(j % 4 + 1) * 64],
                                whT[:, (k * 8 + j) * 128:(k * 8 + j + 1) * 128],
                                hT[:, k * 64:(k + 1) * 64],
                                start=False, stop=(k == 1),
                                skip_group_check=True)

                # ---- gates (transposed: partitions=features, cols=batch) ----
                sig_fi = gates.tile([128, 256], BF16, tag="sfi", name="sig_fi")
                nc.scalar.activation(sig_fi, pfi, SIGM)
                tg = gates.tile([128, 128], BF16, tag="tg", name="tg")
                nc.scalar.activation(tg, pgo[:, 0:128], TANH)
                sig_o = gates.tile([128, 128], BF16, tag="so", name="sig_o")
                nc.scalar.activation(sig_o, pgo[:, 128:256], SIGM)

                # next step's x-part: the PE runs it while ACT/DVE work.
                if t + 1 < T:
                    pz_next = x_mms(t + 1, stop=False)

                # ---- dense (Wd) accumulation for this timestep ----
                wds_off = (t % 8) * 256
                for m in range(2):
                    nc.tensor.matmul(
                        out2T[:, m * 64:(m + 1) * 64],
                        wds[:, wds_off + m * 128:wds_off + (m + 1) * 128],
                        convT[:, tb],
                        start=(t == 0 and m == 0), stop=(t == T - 1),
                        skip_group_check=True)

                # prefetch next conv chunk (PE matmul runs in the idle gap)
                if t % 8 == 0 and t + 8 < T:
                    pcv_cur, wds_cur = conv_chunk(t // 8 + 1)

                # ---- c update (all-bf16 so the DVE runs in 2x mode) ----
                c_new = state.tile([128, 128], BF16, tag="c", name="c_new")
                if t == 0:
                    nc.vector.tensor_tensor(out=c_new, in0=sig_fi[:, 128:256],
                                            in1=tg, op=MUL)
                else:
                    q1 = gates.tile([128, 128], BF16, tag="q1", name="q1")
                    nc.vector.tensor_tensor(out=q1, in0=sig_fi[:, 0:128],
                                            in1=cT, op=MUL)
                    q2 = gates.tile([128, 128], BF16, tag="q2", name="q2")
                    nc.vector.tensor_tensor(out=q2, in0=sig_fi[:, 128:256],
                                            in1=tg, op=MUL)
                    nc.vector.tensor_tensor(out=c_new, in0=q1, in1=q2, op=ADD)
                cT = c_new

                tcT = gates.tile([128, 128], BF16, tag="tc", name="tcT")
                nc.scalar.activation(tcT, c_new, TANH)

                # conv lrelu halves ride the ACT idle window after tanh(c)
                if t % 8 in (0, 1) and t - (t % 8) + 8 < T:
                    conv_lrelu(pcv_cur, t // 8 + 1, t % 8)

                h_new = state.tile([128, 128], BF16, tag="h", name="h_new")
                nc.vector.tensor_tensor(out=h_new, in0=sig_o, in1=tcT, op=MUL)
                hT = h_new

                if t + 1 < T:
                    pfi, pgo = pz_next

            # ---- u^T tiles (bf16): [lrelu(h) ; lrelu(out2+bd)] ----
            # uh on the DVE (lrelu = (h*0.2) max h) so it doesn't pay the
            # ACT round-trip right after the last step's h.
            uh = const.tile([128, 128], BF16)
            nc.vector.scalar_tensor_tensor(
                out=uh, in0=hT, scalar=0.2, in1=hT,
                op0=mybir.AluOpType.mult, op1=mybir.AluOpType.max)
            uo = const.tile([128, 128], BF16)
            nc.scalar.activation(uo[:, 0:64], out2T[:, 0:64], PRELU,
                                 bias=bdp[:, 0:1], alpha=0.2)
            nc.scalar.activation(uo[:, 64:128], out2T[:, 64:128], PRELU,
                                 bias=bdp[:, 1:2], alpha=0.2)

            # ---- MLP in transposed orientation ----
            # Biases ride as an extra contraction row (b*bp row 0, onesp
            # rhs), so the PRELU needs no per-chunk bias and 4 m-chunks
            # share one wide ACT op.
            # m1T/m2T as half-tiles so consumers of the first half do not
            # wait on the second half's ACT (dep tracking is tile-granular
            # for these ACT writes).
            m1T = [const.tile([128, 256], BF16, name="m1a"),
                   const.tile([128, 256], BF16, name="m1b")]
            for half in range(2):
                pm = ps_z.tile([128, 256], F32, tag="pz", name="pm")
                for mc in range(4):
                    m = half * 4 + mc
                    ms = slice(mc * 64, (mc + 1) * 64)
                    # k=2,3 (dense-branch inputs) first: they are ready
                    # before the last LSTM step's h.
                    for j, k in enumerate((2, 3, 0, 1)):
                        u_src = uo if k >= 2 else uh
                        nc.tensor.matmul(pm[:, ms],
                                         w1p[:, (k * 8 + m) * 128:(k * 8 + m + 1) * 128],
                                         u_src[:, (k % 2) * 64:(k % 2 + 1) * 64],
                                         start=(mc == 0 and j == 0), stop=False,
                                         skip_group_check=True)
                    nc.tensor.matmul(pm[:, ms], b1bp[:, m * 128:(m + 1) * 128],
                                     onesp, start=False, stop=True,
                                     skip_group_check=True)
                nc.scalar.activation(m1T[half], pm, PRELU, alpha=0.3)
            m2T = [const.tile([128, 256], BF16, name="m2a"),
                   const.tile([128, 256], BF16, name="m2b")]
            for half in range(2):
                pm = ps_z.tile([128, 256], F32, tag="pz", name="pm")
                for mc in range(4):
                    m = half * 4 + mc
                    ms = slice(mc * 64, (mc + 1) * 64)
                    for k in range(8):
                        nc.tensor.matmul(pm[:, ms],
                                         w2p[:, (k * 8 + m) * 128:(k * 8 + m + 1) * 128],
                                         m1T[k // 4][:, (k % 4) * 64:(k % 4 + 1) * 64],
                                         start=(mc == 0 and k == 0), stop=False,
                                         skip_group_check=True)
                    nc.tensor.matmul(pm[:, ms], b2bp[:, m * 128:(m + 1) * 128],
                                     onesp, start=False, stop=True,
                                     skip_group_check=True)
                nc.scalar.activation(m2T[half], pm, PRELU, alpha=0.3)
            po = ps_cv.tile([1, 64], F32, tag="cv", name="po")
            for k in range(8):
                nc.tensor.matmul(po, w3p[:, k:k + 1],
                                 m2T[k // 4][:, (k % 4) * 64:(k % 4 + 1) * 64],
                                 start=(k == 0), stop=(k == 7))
            oS = gates.tile([1, 64], F32, tag="oS", name="oS")
            nc.scalar.activation(oS, po, SIGM, bias=b3p)
            nc.sync.dma_start(out=out_d.rearrange("a b -> b a"), in_=oS)

    nc.compile()
    return nc


def _prep_weights(inputs):
    """Host-side packing of all weights (shared across cores)."""
    Wx = np.asarray(inputs["Wx"], np.float32)
    Wh = np.asarray(inputs["Wh"], np.float32)
    b_lstm = np.asarray(inputs["b_lstm"], np.float32)
    Wc = np.asarray(inputs["Wc"], np.float32)
    bc = np.asarray(inputs["bc"], np.float32)
    Wd = np.asarray(inputs["Wd"], np.float32)
    bd = np.asarray(inputs["bd"], np.float32)
    W1 = np.asarray(inputs["W1"], np.float32)
    b1 = np.asarray(inputs["b1"], np.float32)
    W2 = np.asarray(inputs["W2"], np.float32)
    b2 = np.asarray(inputs["b2"], np.float32)
    W3 = np.asarray(inputs["W3"], np.float32)
    b3 = np.asarray(inputs["b3"], np.float32)

    BFD = np.dtype("bfloat16")

    # gate column permutation i f g o -> f i g o
    perm = np.concatenate([np.arange(256, 512), np.arange(0, 256),
                           np.arange(512, 1024)])

    wxT = np.zeros((128, 4 * H), np.float32)
    wxT[0:F] = Wx[:, perm]
    wxT[F] = b_lstm[perm]
    wxT = wxT.astype(BFD)

    whp_n = Wh[:, perm]
    whT = np.concatenate([whp_n[0:128], whp_n[128:256]], axis=1).astype(BFD)

    wcp = np.zeros((128, CF), np.float32)
    wcp[0:F] = Wc
    wcp[F] = bc
    wcp = wcp.astype(BFD)

    # Wd: (T*CF, H) -> per (t, m) chunk (128cf x 128h)
    wd4 = Wd.reshape(T, CF, 2, 128)          # t, cf, m, j
    wdp = np.ascontiguousarray(
        wd4.transpose(1, 0, 2, 3).reshape(128, T * 2 * 128)
    ).astype(BFD)
    bdp = np.ascontiguousarray(bd.reshape(2, 128).T)  # (128, 2)

    # fold BN1/BN2/BN3 into W1/b1
    a1 = inputs["bn1_g"] / np.sqrt(inputs["bn1_v"] + EPS)
    o1 = inputs["bn1_b"] - inputs["bn1_m"] * a1
    a2 = inputs["bn2_g"] / np.sqrt(inputs["bn2_v"] + EPS)
    o2 = inputs["bn2_b"] - inputs["bn2_m"] * a2
    a3 = inputs["bn3_g"] / np.sqrt(inputs["bn3_v"] + EPS)
    o3 = inputs["bn3_b"] - inputs["bn3_m"] * a3
    A = np.asarray(a3 * np.concatenate([a1, a2]), np.float32)       # (512,)
    Boff = np.asarray(a3 * np.concatenate([o1, o2]) + o3, np.float32)
    W1f = (A[:, None] * W1).astype(np.float32)
    b1f = (Boff @ W1 + b1).astype(np.float32)

    def pack_T(w, kc, mc):
        # (kc*128, mc*128) -> (128, kc*mc*128), chunk (k,m) at [(k*mc+m)*128]
        return np.ascontiguousarray(
            w.reshape(kc, 128, mc, 128).transpose(1, 0, 2, 3).reshape(128, kc * mc * 128)
        )

    w1p = pack_T(W1f, 4, 8).astype(BFD)
    b1bp = np.zeros((128, 8 * 128), np.float32)
    b1bp[0] = b1f
    b1bp = b1bp.astype(BFD)
    w2p = pack_T(W2, 8, 8).astype(BFD)
    b2bp = np.zeros((128, 8 * 128), np.float32)
    b2bp[0] = b2
    b2bp = b2bp.astype(BFD)
    w3p = np.ascontiguousarray(W3.reshape(8, 128, 1)[:, :, 0].T).astype(BFD)
    b3p = b3.reshape(1, 1)
    onesp = np.zeros((128, 64), np.float32)
    onesp[0] = 1.0
    onesp = onesp.astype(BFD)

    return dict(wxT=wxT, whT=np.ascontiguousarray(whT), wcp=wcp, wdp=wdp,
                bdp=bdp, w1p=w1p, b1bp=b1bp, w2p=w2p, b2bp=b2bp, w3p=w3p,
                b3p=b3p, onesp=onesp)


def kernel(**inputs):
    if "nc" not in _CACHE:
        _CACHE["nc"] = _build_nc()
    nc = _CACHE["nc"]

    x = np.asarray(inputs["inputs"], np.float32)  # (B, T, F)
    w = _prep_weights(inputs)

    in_maps = []
    for c in range(N_CORES):
        xc = x[c * BC:(c + 1) * BC]               # (BC, T, F)
        xT = np.zeros((128, NT), np.float32)
        xT[0:F] = xc.transpose(2, 1, 0).reshape(F, NT)  # [f, t*BC+b]
        xT[F] = 1.0
        in_maps.append({"xT": xT.astype(np.dtype("bfloat16")), **w})

    res = bass_utils.run_bass_kernel_spmd(nc, in_maps, core_ids=list(range(N_CORES)))
    out = np.concatenate([res.results[c]["out"] for c in range(N_CORES)], axis=0)
    return out.astype(np.float32)
